# revision 2
# baseline (speedup 1.0000x reference)
"""VQ codebook kernel for 8 Trainium2 NeuronCores (data-parallel over batch).

Problem (hardcoded shapes):
  x            [64, 256, 32, 32] f32
  lookup_table [512, 256] f32
  outputs: x_e [64, 256, 32, 32] f32, q_x [64, 32, 32] int32, loss scalar f32

Algorithm per core (8 batches of x, i.e. 8192 pixels):
  - x[b] in natural layout is [d=256, hw=1024]  -> already the matmul lhsT
    (contraction over d as the partition dim, 2 chunks of 128).
  - t = x^T @ ltt - cn/2  (ltt = codebook^T staged [256,512]; cn = row norms,
    host-precomputed in float64, passed as negcn = -cn/2). argmax_k t ==
    argmin_k ||x - c_k||^2, ties -> first index (matches jnp.argmin).
  - per 128-pixel tile: 2 accumulating matmuls -> PSUM [128,512]; DVE adds
    negcn; DVE reduce_max + max_index give maxval/argmax; GPSIMD indirect
    DMA gathers codebook rows -> x_e rows; stores are contiguous row-major.
  - loss = 1.25 * (sum(x^2) - 2*sum(maxval)) / (N*D); sum(x^2) accumulated
    on the Scalar engine (Square activation with accum_out), per-core
    partials [128,1] are summed on host (the scalar-loss all-reduce).
"""

from contextlib import ExitStack

import numpy as np

import concourse.bass as bass
import concourse.mybir as mybir
from concourse import bacc
from concourse.tile import TileContext
from concourse.bass_utils import run_bass_kernel_spmd

P = 128
K = 512
DIM = 256
B = 64
HW = 1024  # 32*32
NCORES = 8
BPC = B // NCORES  # batches per core
NPC = BPC * HW     # pixels per core
BETA = 0.25

# "fp32": exact 4-pass fp32 matmuls. "f32r": 1-pass FP22-truncated matmuls.
MM_MODE = "fp32"

F32 = mybir.dt.float32
F32R = mybir.dt.float32r
U32 = mybir.dt.uint32


def _mm_ap(ap):
    if MM_MODE == "f32r":
        return ap.bitcast(F32R)
    return ap


def build_kernel():
    nc = bacc.Bacc(
        "TRN2",
        target_bir_lowering=False,
        debug=False,
        enable_asserts=False,
        num_devices=NCORES,
    )
    x = nc.dram_tensor("x", [BPC, DIM, HW], F32, kind="ExternalInput").ap()
    ltt = nc.dram_tensor("ltt", [DIM, K], F32, kind="ExternalInput").ap()
    lt = nc.dram_tensor("lt", [K, DIM], F32, kind="ExternalInput").ap()
    negcn = nc.dram_tensor("negcn", [1, K], F32, kind="ExternalInput").ap()
    xe = nc.dram_tensor("xe", [NPC, DIM], F32, kind="ExternalOutput").ap()
    q = nc.dram_tensor("q", [NPC, 1], U32, kind="ExternalOutput").ap()
    lsum = nc.dram_tensor("lsum", [P, 1], F32, kind="ExternalOutput").ap()

    with TileContext(nc) as tc, ExitStack() as ctx:
        const = ctx.enter_context(tc.tile_pool(name="const", bufs=1))
        xpool = ctx.enter_context(tc.tile_pool(name="xp", bufs=2))
        tpool = ctx.enter_context(tc.tile_pool(name="tp", bufs=4))
        gpool = ctx.enter_context(tc.tile_pool(name="gp", bufs=4))
        ipool = ctx.enter_context(tc.tile_pool(name="ip", bufs=4))
        spool = ctx.enter_context(tc.tile_pool(name="sp", bufs=2))
        psum = ctx.enter_context(tc.tile_pool(name="ps", bufs=4, space="PSUM"))

        # --- constants ---
        ltt_sb = const.tile([P, 2 * K], F32)  # [:, :K] = d 0..127, [:, K:] = d 128..255
        nc.sync.dma_start(out=ltt_sb[:, 0:K], in_=ltt[0:P, :])
        nc.sync.dma_start(out=ltt_sb[:, K : 2 * K], in_=ltt[P : 2 * P, :])
        negcn_b = const.tile([P, K], F32)
        nc.sync.dma_start(out=negcn_b[:], in_=negcn[:].to_broadcast([P, K]))

        acc_mv = const.tile([P, 8 * BPC], F32)    # per-tile max values
        acc_xsq = const.tile([P, 2 * BPC], F32)   # per-chunk sum(x^2)

        for b in range(BPC):
            xb = xpool.tile([P, 2 * HW], F32)
            nc.sync.dma_start(out=xb[:, 0:HW], in_=x[b, 0:P, :])
            nc.sync.dma_start(out=xb[:, HW : 2 * HW], in_=x[b, P : 2 * P, :])

            for c in range(2):
                scr = spool.tile([P, HW], F32)
                nc.scalar.activation(
                    out=scr[:],
                    in_=xb[:, c * HW : (c + 1) * HW],
                    func=mybir.ActivationFunctionType.Square,
                    accum_out=acc_xsq[:, 2 * b + c : 2 * b + c + 1],
                )

            for ti in range(8):
                col = b * 8 + ti
                n0 = ti * P
                pt = psum.tile([P, K], F32)
                nc.tensor.matmul(
                    out=pt[:],
                    lhsT=_mm_ap(xb[:, n0 : n0 + P]),
                    rhs=_mm_ap(ltt_sb[:, 0:K]),
                    start=True,
                    stop=False,
                )
                nc.tensor.matmul(
                    out=pt[:],
                    lhsT=_mm_ap(xb[:, HW + n0 : HW + n0 + P]),
                    rhs=_mm_ap(ltt_sb[:, K : 2 * K]),
                    start=False,
                    stop=True,
                )
                t_sb = tpool.tile([P, K], F32)
                nc.vector.tensor_tensor(
                    out=t_sb[:], in0=pt[:], in1=negcn_b[:], op=mybir.AluOpType.add
                )
                nc.vector.reduce_max(
                    out=acc_mv[:, col : col + 1], in_=t_sb[:], axis=mybir.AxisListType.X
                )
                idx8 = ipool.tile([P, 8], U32)
                nc.vector.max_index(
                    out=idx8[:],
                    in_max=acc_mv[:, col : col + 1].to_broadcast([P, 8]),
                    in_values=t_sb[:],
                )
                g = gpool.tile([P, DIM], F32)
                nc.gpsimd.indirect_dma_start(
                    out=g[:],
                    out_offset=None,
                    in_=lt[:, :],
                    in_offset=bass.IndirectOffsetOnAxis(ap=idx8[:, 0:1], axis=0),
                )
                n_glob = b * HW + n0
                nc.sync.dma_start(out=xe[n_glob : n_glob + P, :], in_=g[:])
                nc.sync.dma_start(out=q[n_glob : n_glob + P, :], in_=idx8[:, 0:1])

        # --- loss partials: lsum = sum(x^2) - 2*sum(maxval), per partition ---
        r1 = const.tile([P, 1], F32)
        r2 = const.tile([P, 1], F32)
        lso = const.tile([P, 1], F32)
        nc.vector.reduce_sum(out=r1[:], in_=acc_mv[:], axis=mybir.AxisListType.X)
        nc.vector.reduce_sum(out=r2[:], in_=acc_xsq[:], axis=mybir.AxisListType.X)
        nc.vector.tensor_scalar_mul(out=r1[:], in0=r1[:], scalar1=-2.0)
        nc.vector.tensor_add(out=lso[:], in0=r1[:], in1=r2[:])
        nc.sync.dma_start(out=lsum[:], in_=lso[:])

    nc.finalize()
    return nc


_NC_CACHE = {}


def _get_nc():
    key = MM_MODE
    if key not in _NC_CACHE:
        _NC_CACHE[key] = build_kernel()
    return _NC_CACHE[key]


def _prepare_in_maps(x, lookup_table):
    x = np.asarray(x, dtype=np.float32).reshape(B, DIM, HW)
    lt = np.ascontiguousarray(np.asarray(lookup_table, dtype=np.float32))
    ltt = np.ascontiguousarray(lt.T)
    cn = np.sum(lt.astype(np.float64) ** 2, axis=1)
    negcn = (-0.5 * cn).astype(np.float32).reshape(1, K)
    in_maps = []
    for c in range(NCORES):
        in_maps.append(
            {
                "x": np.ascontiguousarray(x[c * BPC : (c + 1) * BPC]),
                "ltt": ltt,
                "lt": lt,
                "negcn": negcn,
            }
        )
    return in_maps


def _assemble(results):
    xe_parts = []
    q_parts = []
    total = 0.0
    for r in results:
        xe_rows = np.asarray(r["xe"])  # [NPC, DIM]
        q_rows = np.asarray(r["q"]).reshape(NPC)
        lsum = np.asarray(r["lsum"], dtype=np.float64)
        xe_parts.append(
            xe_rows.reshape(BPC, 32, 32, DIM).transpose(0, 3, 1, 2)
        )
        q_parts.append(q_rows.reshape(BPC, 32, 32).astype(np.int32))
        total += float(lsum.sum())
    x_e = np.ascontiguousarray(np.concatenate(xe_parts, axis=0), dtype=np.float32)
    q_x = np.ascontiguousarray(np.concatenate(q_parts, axis=0))
    loss = np.float32((1.0 + BETA) * total / (B * DIM * HW))
    return x_e, q_x, loss


def run(x, lookup_table, trace=False):
    nc = _get_nc()
    in_maps = _prepare_in_maps(x, lookup_table)
    res = run_bass_kernel_spmd(nc, in_maps, core_ids=list(range(NCORES)), trace=trace)
    return _assemble(res.results), res


def kernel(x, lookup_table):
    (x_e, q_x, loss), _ = run(x, lookup_table, trace=False)
    return x_e, q_x, loss


# revision 8
# speedup vs baseline: 1.4884x; 1.4884x over previous
"""VQ codebook kernel for 8 Trainium2 NeuronCores (data-parallel over batch).

Problem (hardcoded shapes):
  x            [64, 256, 32, 32] f32
  lookup_table [512, 256] f32
  outputs: x_e [64, 256, 32, 32] f32, q_x [64, 32, 32] int32, loss scalar f32

Algorithm per core (8 batches of x, i.e. 8192 pixels):
  - x[b] in natural layout is [d=256, hw=1024]  -> already the matmul lhsT
    (contraction over d as the partition dim, 2 chunks of 128).
  - t = x^T @ ltt - cn/2  (ltt = codebook^T staged [256,512]; cn = row norms,
    host-precomputed in float64, passed as negcn = -cn/2). argmax_k t ==
    argmin_k ||x - c_k||^2, ties -> first index (matches jnp.argmin).
  - per 128-pixel tile: 2 accumulating matmuls -> PSUM [128,512]; DVE adds
    negcn; DVE reduce_max + max_index give maxval/argmax; GPSIMD indirect
    DMA gathers codebook rows -> x_e rows; stores are contiguous row-major.
  - loss = 1.25 * (sum(x^2) - 2*sum(maxval)) / (N*D); sum(x^2) accumulated
    on the Scalar engine (Square activation with accum_out), per-core
    partials [128,1] are summed on host (the scalar-loss all-reduce).
"""

from contextlib import ExitStack

import numpy as np

import concourse.bass as bass
import concourse.mybir as mybir
from concourse import bacc
from concourse.tile import TileContext
from concourse.bass_utils import run_bass_kernel_spmd

P = 128
K = 512
DIM = 256
B = 64
HW = 1024  # 32*32
NCORES = 8
BPC = B // NCORES  # batches per core
NPC = BPC * HW     # pixels per core
BETA = 0.25

# "fp32": exact 4-pass fp32 matmuls. "f32r": 1-pass FP22-truncated matmuls.
MM_MODE = "fp32"

F32 = mybir.dt.float32
F32R = mybir.dt.float32r
U32 = mybir.dt.uint32


def _mm_ap(ap):
    if MM_MODE == "f32r":
        return ap.bitcast(F32R)
    return ap


def build_kernel():
    nc = bacc.Bacc(
        "TRN2",
        target_bir_lowering=False,
        debug=False,
        enable_asserts=False,
        num_devices=NCORES,
    )
    x = nc.dram_tensor("x", [BPC, DIM, HW], F32, kind="ExternalInput").ap()
    ltt = nc.dram_tensor("ltt", [DIM, K], F32, kind="ExternalInput").ap()
    lt = nc.dram_tensor("lt", [K, DIM], F32, kind="ExternalInput").ap()
    negcn = nc.dram_tensor("negcn", [1, K], F32, kind="ExternalInput").ap()
    xe = nc.dram_tensor("xe", [NPC, DIM], F32, kind="ExternalOutput").ap()
    # q[p, col] = argmax index of pixel n = col*128 + p (host transposes)
    q = nc.dram_tensor("q", [P, 8 * BPC], U32, kind="ExternalOutput").ap()
    lsum = nc.dram_tensor("lsum", [P, 1], F32, kind="ExternalOutput").ap()

    with TileContext(nc) as tc, ExitStack() as ctx:
        const = ctx.enter_context(tc.tile_pool(name="const", bufs=1))
        xpool = ctx.enter_context(tc.tile_pool(name="xp", bufs=2))
        tpool = ctx.enter_context(tc.tile_pool(name="tp", bufs=4))
        gpool = ctx.enter_context(tc.tile_pool(name="gp", bufs=4))
        ipool = ctx.enter_context(tc.tile_pool(name="ip", bufs=4))
        spool = ctx.enter_context(tc.tile_pool(name="sp", bufs=3))
        psum = ctx.enter_context(tc.tile_pool(name="ps", bufs=6, space="PSUM"))

        # --- constants ---
        ltt_sb = const.tile([P, 2 * K], F32)  # [:, :K] = d 0..127, [:, K:] = d 128..255
        nc.sync.dma_start(out=ltt_sb[:, 0:K], in_=ltt[0:P, :])
        nc.sync.dma_start(out=ltt_sb[:, K : 2 * K], in_=ltt[P : 2 * P, :])
        negcn_b = const.tile([P, K], F32)
        nc.sync.dma_start(out=negcn_b[:], in_=negcn[:].to_broadcast([P, K]))

        acc_mv = const.tile([P, 8 * BPC], F32)    # per-tile max values
        acc_xsq = const.tile([P, 2 * BPC], F32)   # per-chunk sum(x^2)
        acc_q8 = const.tile([P, 8 * 8 * BPC], U32)  # per-tile max_index outputs (8 wide)

        for b in range(BPC):
            xb = xpool.tile([P, 2 * HW], F32)
            nc.sync.dma_start(out=xb[:, 0:HW], in_=x[b, 0:P, :])
            nc.sync.dma_start(out=xb[:, HW : 2 * HW], in_=x[b, P : 2 * P, :])

            for c in range(2):
                scr = spool.tile([P, HW], F32)
                nc.scalar.activation(
                    out=scr[:],
                    in_=xb[:, c * HW : (c + 1) * HW],
                    func=mybir.ActivationFunctionType.Square,
                    accum_out=acc_xsq[:, 2 * b + c : 2 * b + c + 1],
                )

            for ti in range(8):
                col = b * 8 + ti
                n0 = ti * P
                pt = psum.tile([P, K], F32)
                nc.tensor.matmul(
                    out=pt[:],
                    lhsT=_mm_ap(xb[:, n0 : n0 + P]),
                    rhs=_mm_ap(ltt_sb[:, 0:K]),
                    start=True,
                    stop=False,
                )
                nc.tensor.matmul(
                    out=pt[:],
                    lhsT=_mm_ap(xb[:, HW + n0 : HW + n0 + P]),
                    rhs=_mm_ap(ltt_sb[:, K : 2 * K]),
                    start=False,
                    stop=True,
                )
                # Scalar engine drains PSUM (idle otherwise); DVE then runs
                # add/max/max_index from SBUF at 2x where supported.
                s_sb = spool.tile([P, K], F32, tag="scopy")
                nc.scalar.copy(out=s_sb[:], in_=pt[:])
                t_sb = tpool.tile([P, K], F32)
                nc.vector.tensor_tensor(
                    out=t_sb[:], in0=s_sb[:], in1=negcn_b[:], op=mybir.AluOpType.add
                )
                nc.vector.reduce_max(
                    out=acc_mv[:, col : col + 1], in_=t_sb[:], axis=mybir.AxisListType.X
                )
                idx8 = acc_q8[:, 8 * col : 8 * col + 8]
                nc.vector.max_index(
                    out=idx8,
                    in_max=acc_mv[:, col : col + 1].to_broadcast([P, 8]),
                    in_values=t_sb[:],
                )
                g = gpool.tile([P, DIM], F32)
                nc.gpsimd.indirect_dma_start(
                    out=g[:],
                    out_offset=None,
                    in_=lt[:, :],
                    in_offset=bass.IndirectOffsetOnAxis(ap=idx8[:, 0:1], axis=0),
                )
                n_glob = b * HW + n0
                nc.sync.dma_start(out=xe[n_glob : n_glob + P, :], in_=g[:])

        # --- single batched q store (column 0 of each 8-wide max_index group) ---
        acc_q8v = acc_q8[:].rearrange("p (c e) -> p c e", e=8)
        nc.sync.dma_start(out=q[:, :], in_=acc_q8v[:, :, 0:1])

        # --- loss partials: lsum = sum(x^2) - 2*sum(maxval), per partition ---
        r1 = const.tile([P, 1], F32)
        r2 = const.tile([P, 1], F32)
        lso = const.tile([P, 1], F32)
        nc.vector.reduce_sum(out=r1[:], in_=acc_mv[:], axis=mybir.AxisListType.X)
        nc.vector.reduce_sum(out=r2[:], in_=acc_xsq[:], axis=mybir.AxisListType.X)
        nc.vector.tensor_scalar_mul(out=r1[:], in0=r1[:], scalar1=-2.0)
        nc.vector.tensor_add(out=lso[:], in0=r1[:], in1=r2[:])
        nc.sync.dma_start(out=lsum[:], in_=lso[:])

    nc.finalize()
    return nc


_NC_CACHE = {}


def _get_nc():
    key = MM_MODE
    if key not in _NC_CACHE:
        _NC_CACHE[key] = build_kernel()
    return _NC_CACHE[key]


def _prepare_in_maps(x, lookup_table):
    x = np.asarray(x, dtype=np.float32).reshape(B, DIM, HW)
    lt = np.ascontiguousarray(np.asarray(lookup_table, dtype=np.float32))
    ltt = np.ascontiguousarray(lt.T)
    cn = np.sum(lt.astype(np.float64) ** 2, axis=1)
    negcn = (-0.5 * cn).astype(np.float32).reshape(1, K)
    in_maps = []
    for c in range(NCORES):
        in_maps.append(
            {
                "x": np.ascontiguousarray(x[c * BPC : (c + 1) * BPC]),
                "ltt": ltt,
                "lt": lt,
                "negcn": negcn,
            }
        )
    return in_maps


def _assemble(results):
    xe_parts = []
    q_parts = []
    total = 0.0
    for r in results:
        xe_rows = np.asarray(r["xe"])  # [NPC, DIM]
        q_rows = np.asarray(r["q"]).T.reshape(NPC)  # [P, 64] -> n = col*128 + p
        lsum = np.asarray(r["lsum"], dtype=np.float64)
        xe_parts.append(
            xe_rows.reshape(BPC, 32, 32, DIM).transpose(0, 3, 1, 2)
        )
        q_parts.append(q_rows.reshape(BPC, 32, 32).astype(np.int32))
        total += float(lsum.sum())
    x_e = np.ascontiguousarray(np.concatenate(xe_parts, axis=0), dtype=np.float32)
    q_x = np.ascontiguousarray(np.concatenate(q_parts, axis=0))
    loss = np.float32((1.0 + BETA) * total / (B * DIM * HW))
    return x_e, q_x, loss


def run(x, lookup_table, trace=False):
    nc = _get_nc()
    in_maps = _prepare_in_maps(x, lookup_table)
    res = run_bass_kernel_spmd(nc, in_maps, core_ids=list(range(NCORES)), trace=trace)
    return _assemble(res.results), res


def kernel(x, lookup_table):
    (x_e, q_x, loss), _ = run(x, lookup_table, trace=False)
    return x_e, q_x, loss


# revision 10
# speedup vs baseline: 1.5222x; 1.0227x over previous
"""VQ codebook kernel for 8 Trainium2 NeuronCores (data-parallel over batch).

Problem (hardcoded shapes):
  x            [64, 256, 32, 32] f32
  lookup_table [512, 256] f32
  outputs: x_e [64, 256, 32, 32] f32, q_x [64, 32, 32] int32, loss scalar f32

Algorithm per core (8 batches of x, i.e. 8192 pixels):
  - x[b] in natural layout is [d=256, hw=1024]  -> already the matmul lhsT
    (contraction over d as the partition dim, 2 chunks of 128).
  - t = x^T @ ltt - cn/2  (ltt = codebook^T staged [256,512]; cn = row norms,
    host-precomputed in float64, passed as negcn = -cn/2). argmax_k t ==
    argmin_k ||x - c_k||^2, ties -> first index (matches jnp.argmin).
  - per 128-pixel tile: 2 accumulating matmuls -> PSUM [128,512]; DVE adds
    negcn; DVE reduce_max + max_index give maxval/argmax; GPSIMD indirect
    DMA gathers codebook rows -> x_e rows; stores are contiguous row-major.
  - loss = 1.25 * (sum(x^2) - 2*sum(maxval)) / (N*D); sum(x^2) accumulated
    on the Scalar engine (Square activation with accum_out), per-core
    partials [128,1] are summed on host (the scalar-loss all-reduce).
"""

from contextlib import ExitStack

import numpy as np

import concourse.bass as bass
import concourse.mybir as mybir
from concourse import bacc
from concourse.tile import TileContext
from concourse.bass_utils import run_bass_kernel_spmd

P = 128
K = 512
DIM = 256
B = 64
HW = 1024  # 32*32
NCORES = 8
BPC = B // NCORES  # batches per core
NPC = BPC * HW     # pixels per core
BETA = 0.25

# "fp32": exact 4-pass fp32 matmuls. "f32r": 1-pass FP22-truncated matmuls.
MM_MODE = "fp32"

F32 = mybir.dt.float32
F32R = mybir.dt.float32r
U32 = mybir.dt.uint32


def _mm_ap(ap):
    if MM_MODE == "f32r":
        return ap.bitcast(F32R)
    return ap


def build_kernel():
    nc = bacc.Bacc(
        "TRN2",
        target_bir_lowering=False,
        debug=False,
        enable_asserts=False,
        num_devices=NCORES,
    )
    x = nc.dram_tensor("x", [BPC, DIM, HW], F32, kind="ExternalInput").ap()
    ltt = nc.dram_tensor("ltt", [DIM, K], F32, kind="ExternalInput").ap()
    lt = nc.dram_tensor("lt", [K, DIM], F32, kind="ExternalInput").ap()
    negcn = nc.dram_tensor("negcn", [1, K], F32, kind="ExternalInput").ap()
    xe = nc.dram_tensor("xe", [NPC, DIM], F32, kind="ExternalOutput").ap()
    # q[p, col] = argmax index of pixel n = col*128 + p (host transposes)
    q = nc.dram_tensor("q", [P, 8 * BPC], U32, kind="ExternalOutput").ap()
    lsum = nc.dram_tensor("lsum", [P, 1], F32, kind="ExternalOutput").ap()

    with TileContext(nc) as tc, ExitStack() as ctx:
        const = ctx.enter_context(tc.tile_pool(name="const", bufs=1))
        xpool = ctx.enter_context(tc.tile_pool(name="xp", bufs=4))
        tpool = ctx.enter_context(tc.tile_pool(name="tp", bufs=6))
        gpool = ctx.enter_context(tc.tile_pool(name="gp", bufs=6))
        ipool = ctx.enter_context(tc.tile_pool(name="ip", bufs=4))
        spool = ctx.enter_context(tc.tile_pool(name="sp", bufs=4))
        psum = ctx.enter_context(tc.tile_pool(name="ps", bufs=6, space="PSUM"))

        # --- constants ---
        ltt_sb = const.tile([P, 2 * K], F32)  # [:, :K] = d 0..127, [:, K:] = d 128..255
        nc.sync.dma_start(out=ltt_sb[:, 0:K], in_=ltt[0:P, :])
        nc.sync.dma_start(out=ltt_sb[:, K : 2 * K], in_=ltt[P : 2 * P, :])
        negcn_b = const.tile([P, K], F32)
        nc.sync.dma_start(out=negcn_b[:], in_=negcn[:].to_broadcast([P, K]))

        acc_mv = const.tile([P, 8 * BPC], F32)    # per-tile max values
        acc_xsq = const.tile([P, 2 * BPC], F32)   # per-chunk sum(x^2)
        acc_q8 = const.tile([P, 8 * 8 * BPC], U32)  # per-tile max_index outputs (8 wide)

        for b in range(BPC):
            xb = xpool.tile([P, 2 * HW], F32)
            nc.sync.dma_start(out=xb[:, 0:HW], in_=x[b, 0:P, :])
            nc.sync.dma_start(out=xb[:, HW : 2 * HW], in_=x[b, P : 2 * P, :])

            for c in range(2):
                scr = spool.tile([P, HW], F32)
                nc.scalar.activation(
                    out=scr[:],
                    in_=xb[:, c * HW : (c + 1) * HW],
                    func=mybir.ActivationFunctionType.Square,
                    accum_out=acc_xsq[:, 2 * b + c : 2 * b + c + 1],
                )

            for ti in range(8):
                col = b * 8 + ti
                n0 = ti * P
                pt = psum.tile([P, K], F32)
                nc.tensor.matmul(
                    out=pt[:],
                    lhsT=_mm_ap(xb[:, n0 : n0 + P]),
                    rhs=_mm_ap(ltt_sb[:, 0:K]),
                    start=True,
                    stop=False,
                )
                nc.tensor.matmul(
                    out=pt[:],
                    lhsT=_mm_ap(xb[:, HW + n0 : HW + n0 + P]),
                    rhs=_mm_ap(ltt_sb[:, K : 2 * K]),
                    start=False,
                    stop=True,
                )
                # Scalar engine drains PSUM (idle otherwise); DVE then runs
                # add/max/max_index from SBUF at 2x where supported.
                s_sb = spool.tile([P, K], F32, tag="scopy")
                nc.scalar.copy(out=s_sb[:], in_=pt[:])
                t_sb = tpool.tile([P, K], F32)
                nc.vector.tensor_tensor(
                    out=t_sb[:], in0=s_sb[:], in1=negcn_b[:], op=mybir.AluOpType.add
                )
                nc.vector.reduce_max(
                    out=acc_mv[:, col : col + 1], in_=t_sb[:], axis=mybir.AxisListType.X
                )
                idx8 = acc_q8[:, 8 * col : 8 * col + 8]
                nc.vector.max_index(
                    out=idx8,
                    in_max=acc_mv[:, col : col + 1].to_broadcast([P, 8]),
                    in_values=t_sb[:],
                )
                g = gpool.tile([P, DIM], F32)
                nc.gpsimd.indirect_dma_start(
                    out=g[:],
                    out_offset=None,
                    in_=lt[:, :],
                    in_offset=bass.IndirectOffsetOnAxis(ap=idx8[:, 0:1], axis=0),
                )
                n_glob = b * HW + n0
                # HWDGE via the Scalar sequencer: keeps xe-store descriptor
                # issue off the Sync sequencer, which feeds the x loads.
                nc.scalar.dma_start(out=xe[n_glob : n_glob + P, :], in_=g[:])

        # --- single batched q store (column 0 of each 8-wide max_index group) ---
        acc_q8v = acc_q8[:].rearrange("p (c e) -> p c e", e=8)
        nc.sync.dma_start(out=q[:, :], in_=acc_q8v[:, :, 0:1])

        # --- loss partials: lsum = sum(x^2) - 2*sum(maxval), per partition ---
        r1 = const.tile([P, 1], F32)
        r2 = const.tile([P, 1], F32)
        lso = const.tile([P, 1], F32)
        nc.vector.reduce_sum(out=r1[:], in_=acc_mv[:], axis=mybir.AxisListType.X)
        nc.vector.reduce_sum(out=r2[:], in_=acc_xsq[:], axis=mybir.AxisListType.X)
        nc.vector.tensor_scalar_mul(out=r1[:], in0=r1[:], scalar1=-2.0)
        nc.vector.tensor_add(out=lso[:], in0=r1[:], in1=r2[:])
        nc.sync.dma_start(out=lsum[:], in_=lso[:])

    nc.finalize()
    return nc


_NC_CACHE = {}


def _get_nc():
    key = MM_MODE
    if key not in _NC_CACHE:
        _NC_CACHE[key] = build_kernel()
    return _NC_CACHE[key]


def _prepare_in_maps(x, lookup_table):
    x = np.asarray(x, dtype=np.float32).reshape(B, DIM, HW)
    lt = np.ascontiguousarray(np.asarray(lookup_table, dtype=np.float32))
    ltt = np.ascontiguousarray(lt.T)
    cn = np.sum(lt.astype(np.float64) ** 2, axis=1)
    negcn = (-0.5 * cn).astype(np.float32).reshape(1, K)
    in_maps = []
    for c in range(NCORES):
        in_maps.append(
            {
                "x": np.ascontiguousarray(x[c * BPC : (c + 1) * BPC]),
                "ltt": ltt,
                "lt": lt,
                "negcn": negcn,
            }
        )
    return in_maps


def _assemble(results):
    xe_parts = []
    q_parts = []
    total = 0.0
    for r in results:
        xe_rows = np.asarray(r["xe"])  # [NPC, DIM]
        q_rows = np.asarray(r["q"]).T.reshape(NPC)  # [P, 64] -> n = col*128 + p
        lsum = np.asarray(r["lsum"], dtype=np.float64)
        xe_parts.append(
            xe_rows.reshape(BPC, 32, 32, DIM).transpose(0, 3, 1, 2)
        )
        q_parts.append(q_rows.reshape(BPC, 32, 32).astype(np.int32))
        total += float(lsum.sum())
    x_e = np.ascontiguousarray(np.concatenate(xe_parts, axis=0), dtype=np.float32)
    q_x = np.ascontiguousarray(np.concatenate(q_parts, axis=0))
    loss = np.float32((1.0 + BETA) * total / (B * DIM * HW))
    return x_e, q_x, loss


def run(x, lookup_table, trace=False):
    nc = _get_nc()
    in_maps = _prepare_in_maps(x, lookup_table)
    res = run_bass_kernel_spmd(nc, in_maps, core_ids=list(range(NCORES)), trace=trace)
    return _assemble(res.results), res


def kernel(x, lookup_table):
    (x_e, q_x, loss), _ = run(x, lookup_table, trace=False)
    return x_e, q_x, loss


# revision 16
# speedup vs baseline: 1.5544x; 1.0211x over previous
"""VQ codebook kernel for 8 Trainium2 NeuronCores (data-parallel over batch).

Problem (hardcoded shapes):
  x            [64, 256, 32, 32] f32
  lookup_table [512, 256] f32
  outputs: x_e [64, 256, 32, 32] f32, q_x [64, 32, 32] int32, loss scalar f32

Algorithm per core (8 batches of x, i.e. 8192 pixels):
  - x[b] in natural layout is [d=256, hw=1024]  -> already the matmul lhsT
    (contraction over d as the partition dim, 2 chunks of 128).
  - t = x^T @ ltt - cn/2  (ltt = codebook^T staged [256,512]; cn = row norms,
    host-precomputed in float64, passed as negcn = -cn/2). argmax_k t ==
    argmin_k ||x - c_k||^2, ties -> first index (matches jnp.argmin).
  - per 128-pixel tile: 2 accumulating matmuls -> PSUM [128,512]; DVE adds
    negcn; DVE reduce_max + max_index give maxval/argmax; GPSIMD indirect
    DMA gathers codebook rows -> x_e rows; stores are contiguous row-major.
  - loss = 1.25 * (sum(x^2) - 2*sum(maxval)) / (N*D); sum(x^2) accumulated
    on the Scalar engine (Square activation with accum_out), per-core
    partials [128,1] are summed on host (the scalar-loss all-reduce).
"""

from contextlib import ExitStack

import numpy as np

import concourse.bass as bass
import concourse.mybir as mybir
from concourse import bacc
from concourse.tile import TileContext
from concourse.bass_utils import run_bass_kernel_spmd

P = 128
K = 512
DIM = 256
B = 64
HW = 1024  # 32*32
NCORES = 8
BPC = B // NCORES  # batches per core
NPC = BPC * HW     # pixels per core
BETA = 0.25

# "fp32": exact 4-pass fp32 matmuls. "f32r": 1-pass FP22-truncated matmuls.
MM_MODE = "fp32"

F32 = mybir.dt.float32
F32R = mybir.dt.float32r
U32 = mybir.dt.uint32


def _mm_ap(ap):
    if MM_MODE == "f32r":
        return ap.bitcast(F32R)
    return ap


def build_kernel():
    nc = bacc.Bacc(
        "TRN2",
        target_bir_lowering=False,
        debug=False,
        enable_asserts=False,
        num_devices=NCORES,
    )
    x = nc.dram_tensor("x", [BPC, DIM, HW], F32, kind="ExternalInput").ap()
    ltt = nc.dram_tensor("ltt", [DIM, K], F32, kind="ExternalInput").ap()
    lt = nc.dram_tensor("lt", [K, DIM], F32, kind="ExternalInput").ap()
    negcn = nc.dram_tensor("negcn", [1, K], F32, kind="ExternalInput").ap()
    # p-major per batch: xe[b, p, ti, :] = code vector of pixel n = ti*128 + p
    xe = nc.dram_tensor("xe", [BPC, P, 8, DIM], F32, kind="ExternalOutput").ap()
    # q[p, col] = argmax index of pixel n = col*128 + p (host transposes)
    q = nc.dram_tensor("q", [P, 8 * BPC], U32, kind="ExternalOutput").ap()
    lsum = nc.dram_tensor("lsum", [P, 1], F32, kind="ExternalOutput").ap()

    with TileContext(nc) as tc, ExitStack() as ctx:
        const = ctx.enter_context(tc.tile_pool(name="const", bufs=1))
        xpool = ctx.enter_context(tc.tile_pool(name="xp", bufs=4))
        tpool = ctx.enter_context(tc.tile_pool(name="tp", bufs=6))
        gpool = ctx.enter_context(tc.tile_pool(name="gp", bufs=6))
        ipool = ctx.enter_context(tc.tile_pool(name="ip", bufs=4))
        spool = ctx.enter_context(tc.tile_pool(name="sp", bufs=4))
        psum = ctx.enter_context(tc.tile_pool(name="ps", bufs=6, space="PSUM"))

        # --- constants ---
        ltt_sb = const.tile([P, 2 * K], F32)  # [:, :K] = d 0..127, [:, K:] = d 128..255
        nc.sync.dma_start(out=ltt_sb[:, 0:K], in_=ltt[0:P, :])
        nc.sync.dma_start(out=ltt_sb[:, K : 2 * K], in_=ltt[P : 2 * P, :])
        negcn_b = const.tile([P, K], F32)
        nc.sync.dma_start(out=negcn_b[:], in_=negcn[:].to_broadcast([P, K]))

        acc_mv = const.tile([P, 8 * BPC], F32)    # per-tile max values
        acc_xsq = const.tile([P, 2 * BPC], F32)   # per-chunk sum(x^2)
        acc_q8 = const.tile([P, 8 * 8 * BPC], U32)  # per-tile max_index outputs (8 wide)

        for b in range(BPC):
            g_b = gpool.tile([P, 8 * DIM], F32)
            xb = xpool.tile([P, 2 * HW], F32)
            nc.sync.dma_start(out=xb[:, 0:HW], in_=x[b, 0:P, :])
            nc.sync.dma_start(out=xb[:, HW : 2 * HW], in_=x[b, P : 2 * P, :])

            for c in range(2):
                scr = spool.tile([P, HW], F32)
                nc.scalar.activation(
                    out=scr[:],
                    in_=xb[:, c * HW : (c + 1) * HW],
                    func=mybir.ActivationFunctionType.Square,
                    accum_out=acc_xsq[:, 2 * b + c : 2 * b + c + 1],
                )

            for ti in range(8):
                col = b * 8 + ti
                n0 = ti * P
                pt = psum.tile([P, K], F32)
                nc.tensor.matmul(
                    out=pt[:],
                    lhsT=_mm_ap(xb[:, n0 : n0 + P]),
                    rhs=_mm_ap(ltt_sb[:, 0:K]),
                    start=True,
                    stop=False,
                )
                nc.tensor.matmul(
                    out=pt[:],
                    lhsT=_mm_ap(xb[:, HW + n0 : HW + n0 + P]),
                    rhs=_mm_ap(ltt_sb[:, K : 2 * K]),
                    start=False,
                    stop=True,
                )
                t_sb = tpool.tile([P, K], F32)
                nc.vector.tensor_tensor(
                    out=t_sb[:], in0=pt[:], in1=negcn_b[:], op=mybir.AluOpType.add
                )
                nc.vector.reduce_max(
                    out=acc_mv[:, col : col + 1], in_=t_sb[:], axis=mybir.AxisListType.X
                )
                idx8 = acc_q8[:, 8 * col : 8 * col + 8]
                nc.vector.max_index(
                    out=idx8,
                    in_max=acc_mv[:, col : col + 1].to_broadcast([P, 8]),
                    in_values=t_sb[:],
                )
                nc.gpsimd.indirect_dma_start(
                    out=g_b[:, ti * DIM : (ti + 1) * DIM],
                    out_offset=None,
                    in_=lt[:, :],
                    in_offset=bass.IndirectOffsetOnAxis(ap=idx8[:, 0:1], axis=0),
                )

            nc.scalar.dma_start(out=xe[b], in_=g_b[:])

        # --- single batched q store (column 0 of each 8-wide max_index group) ---
        acc_q8v = acc_q8[:].rearrange("p (c e) -> p c e", e=8)
        nc.sync.dma_start(out=q[:, :], in_=acc_q8v[:, :, 0:1])

        # --- loss partials: lsum = sum(x^2) - 2*sum(maxval), per partition ---
        r1 = const.tile([P, 1], F32)
        r2 = const.tile([P, 1], F32)
        lso = const.tile([P, 1], F32)
        nc.vector.reduce_sum(out=r1[:], in_=acc_mv[:], axis=mybir.AxisListType.X)
        nc.vector.reduce_sum(out=r2[:], in_=acc_xsq[:], axis=mybir.AxisListType.X)
        nc.vector.tensor_scalar_mul(out=r1[:], in0=r1[:], scalar1=-2.0)
        nc.vector.tensor_add(out=lso[:], in0=r1[:], in1=r2[:])
        nc.sync.dma_start(out=lsum[:], in_=lso[:])

    nc.finalize()
    return nc


_NC_CACHE = {}


def _get_nc():
    key = MM_MODE
    if key not in _NC_CACHE:
        _NC_CACHE[key] = build_kernel()
    return _NC_CACHE[key]


def _prepare_in_maps(x, lookup_table):
    x = np.asarray(x, dtype=np.float32).reshape(B, DIM, HW)
    lt = np.ascontiguousarray(np.asarray(lookup_table, dtype=np.float32))
    ltt = np.ascontiguousarray(lt.T)
    cn = np.sum(lt.astype(np.float64) ** 2, axis=1)
    negcn = (-0.5 * cn).astype(np.float32).reshape(1, K)
    in_maps = []
    for c in range(NCORES):
        in_maps.append(
            {
                "x": np.ascontiguousarray(x[c * BPC : (c + 1) * BPC]),
                "ltt": ltt,
                "lt": lt,
                "negcn": negcn,
            }
        )
    return in_maps


def _assemble(results):
    xe_parts = []
    q_parts = []
    total = 0.0
    for r in results:
        xe_d = np.asarray(r["xe"])  # [BPC, P, 8, DIM], pixel n = ti*128 + p
        q_rows = np.asarray(r["q"]).T.reshape(NPC)  # [P, 64] -> n = col*128 + p
        lsum = np.asarray(r["lsum"], dtype=np.float64)
        xe_rows = xe_d.transpose(0, 2, 1, 3)  # [BPC, 8, P, DIM] = n-order
        xe_parts.append(
            xe_rows.reshape(BPC, 32, 32, DIM).transpose(0, 3, 1, 2)
        )
        q_parts.append(q_rows.reshape(BPC, 32, 32).astype(np.int32))
        total += float(lsum.sum())
    x_e = np.ascontiguousarray(np.concatenate(xe_parts, axis=0), dtype=np.float32)
    q_x = np.ascontiguousarray(np.concatenate(q_parts, axis=0))
    loss = np.float32((1.0 + BETA) * total / (B * DIM * HW))
    return x_e, q_x, loss


def run(x, lookup_table, trace=False):
    nc = _get_nc()
    in_maps = _prepare_in_maps(x, lookup_table)
    res = run_bass_kernel_spmd(nc, in_maps, core_ids=list(range(NCORES)), trace=trace)
    return _assemble(res.results), res


def kernel(x, lookup_table):
    (x_e, q_x, loss), _ = run(x, lookup_table, trace=False)
    return x_e, q_x, loss
